# revision 31
# baseline (speedup 1.0000x reference)
"""Trainium2 Bass kernel for nn_ExternalInteraction_9079560863791.

Computes, per batch row b:
    out_user[b, :]  = user_attributes[b, :]  * sum(image_attributes[b, :])
    out_image[b, :] = image_attributes[b, :] * sum(user_attributes[b, :])

Pure data parallel over the batch axis: 2048 rows split across 8 NeuronCores
(256 rows each). Per core: 2 blocks of 128 rows; each block loads a
[128, 4096] f32 tile per tensor, row-sums on the vector engine, and applies
the per-partition broadcast multiply (DVE tensor_scalar for one output, ACT
scaled-copy for the other, to spread compute across engines). Memory-bound:
~16 MiB of HBM traffic per core -> ~47 us roofline at ~358 GB/s.

PRODUCTION PATH = `_build_raw(passes=1)`: a hand-synchronized bacc kernel
(no TileContext). Same body dataflow as the Tile version, but it drops
Tile's fixed per-NEFF overhead — the 21-instruction preamble barrier, the
30-instruction kernel-tail drain + EVSEM butterfly (~9-17 us per the TRN2
docs) — and issues loads on the SP HWDGE queue / stores on the ACT HWDGE
queue. Validated by CoreSim's race detector (which caught the real TRN2
same-engine-RAW pipeline hazard), exact-match vs the Tile kernel on HW,
and 20-exec stress runs.

Measured on hardware (loop/unroll wall-clock differencing; see test.py):
    body steady-state + single-shot:  51-55 us typical, best rounds 43-47
    (device throughput drifts ~+-8 percent between rounds; all sane DMA
     layouts — 1-queue, 2-queue, SWDGE, fused 4 MiB, 1 MiB chunks — are
     statistically indistinguishable within that band)
    theoretical HBM roofline:         46.9 us (358 GB/s/core)
Compute is fully hidden behind DMA. Of note: every DVE op is followed by a
pipeline DRAIN comparable to the op itself, so DVE carries ~41 us/pass of
reduce+mul work — still under the DMA period, but moving *all* compute to
DVE pushes it over (measured 52.8 vs 51.0).

`_build` (Tile) is kept for the For_i timing apparatus: wall-clock slope
over loop iterations isolates on-device time from the ~90-100 ms axon
relay quantum (no NTFF profiling hook exists in this container). Static
large unrolls of the raw kernel are NOT usable for timing — instruction
streaming past IRAM capacity distorts them; the single-pass production
NEFF (~37 instructions/engine) is unaffected.
"""

import sys

for _p in ("/opt/trn_rl_repo", "/opt/pypackages"):
    if _p not in sys.path:
        sys.path.append(_p)

import numpy as np

N_CORES = 8
B, D = 2048, 4096
ROWS = B // N_CORES  # 256 rows per core
P = 128  # SBUF partitions
N_BLOCKS = ROWS // P  # 2 blocks per core

_CACHE = {}


def _build(repeat=1):
    import concourse.tile as tile
    from concourse import bacc, mybir

    nc = bacc.Bacc(
        "TRN2",
        target_bir_lowering=False,
        debug=False,
        enable_asserts=False,
        num_devices=N_CORES,
    )
    f32 = mybir.dt.float32

    u = nc.dram_tensor("user_attributes", [ROWS, D], f32, kind="ExternalInput").ap()
    v = nc.dram_tensor("image_attributes", [ROWS, D], f32, kind="ExternalInput").ap()
    ou = nc.dram_tensor("out_user", [ROWS, D], f32, kind="ExternalOutput").ap()
    ov = nc.dram_tensor("out_image", [ROWS, D], f32, kind="ExternalOutput").ap()

    with tile.TileContext(nc) as tc:
        with (
            tc.tile_pool(name="io", bufs=2) as io_pool,
            tc.tile_pool(name="sums", bufs=2) as sum_pool,
        ):
            for _rep in range(repeat):
                for blk in range(N_BLOCKS):
                    rows = slice(blk * P, (blk + 1) * P)

                    ut = io_pool.tile([P, D], f32, tag="ut")
                    nc.sync.dma_start(ut[:], u[rows, :])
                    vt = io_pool.tile([P, D], f32, tag="vt")
                    nc.sync.dma_start(vt[:], v[rows, :])

                    us = sum_pool.tile([P, 1], f32, tag="us")
                    nc.vector.reduce_sum(us[:], ut[:], axis=mybir.AxisListType.X)
                    vs = sum_pool.tile([P, 1], f32, tag="vs")
                    nc.vector.reduce_sum(vs[:], vt[:], axis=mybir.AxisListType.X)

                    # out_user = user * img_sum on ACT (scaled copy),
                    # out_image = image * usr_sum on DVE (2x tensor_scalar).
                    out_u = io_pool.tile([P, D], f32, tag="out_u")
                    nc.scalar.activation(
                        out_u[:], ut[:], mybir.ActivationFunctionType.Copy, scale=vs[:]
                    )
                    out_v = io_pool.tile([P, D], f32, tag="out_v")
                    nc.vector.tensor_scalar_mul(out_v[:], vt[:], us[:])

                    nc.sync.dma_start(ou[rows, :], out_u[:])
                    nc.sync.dma_start(ov[rows, :], out_v[:])

    nc.compile()
    return nc


def _build_loop(iters, unroll=4, variant="base", bufs=2):
    """Timing-only variant: a For_i loop running the whole pipeline
    iters*unroll times. Used to amplify device time past the ~100 ms axon
    relay quantum so wall-clock differencing can resolve per-pass time."""
    import concourse.tile as tile
    from concourse import bacc, mybir

    nc = bacc.Bacc(
        "TRN2",
        target_bir_lowering=False,
        debug=False,
        enable_asserts=False,
        num_devices=N_CORES,
    )
    f32 = mybir.dt.float32

    u = nc.dram_tensor("user_attributes", [ROWS, D], f32, kind="ExternalInput").ap()
    v = nc.dram_tensor("image_attributes", [ROWS, D], f32, kind="ExternalInput").ap()
    ou = nc.dram_tensor("out_user", [ROWS, D], f32, kind="ExternalOutput").ap()
    ov = nc.dram_tensor("out_image", [ROWS, D], f32, kind="ExternalOutput").ap()

    def body_base(tc, io_pool, sum_pool):
        for blk in range(N_BLOCKS):
            rows = slice(blk * P, (blk + 1) * P)
            ut = io_pool.tile([P, D], f32, tag="ut")
            nc.sync.dma_start(ut[:], u[rows, :])
            vt = io_pool.tile([P, D], f32, tag="vt")
            nc.sync.dma_start(vt[:], v[rows, :])

            us = sum_pool.tile([P, 1], f32, tag="us")
            nc.vector.reduce_sum(us[:], ut[:], axis=mybir.AxisListType.X)
            vs = sum_pool.tile([P, 1], f32, tag="vs")
            nc.vector.reduce_sum(vs[:], vt[:], axis=mybir.AxisListType.X)

            out_u = io_pool.tile([P, D], f32, tag="out_u")
            nc.scalar.activation(
                out_u[:], ut[:], mybir.ActivationFunctionType.Copy, scale=vs[:]
            )
            out_v = io_pool.tile([P, D], f32, tag="out_v")
            nc.vector.tensor_scalar_mul(out_v[:], vt[:], us[:])

            nc.sync.dma_start(ou[rows, :], out_u[:])
            nc.sync.dma_start(ov[rows, :], out_v[:])

    def body_memcpy(tc, io_pool, sum_pool):
        # Same HBM traffic, no compute: ceiling probe for the DMA path.
        for blk in range(N_BLOCKS):
            rows = slice(blk * P, (blk + 1) * P)
            ut = io_pool.tile([P, D], f32, tag="ut")
            nc.sync.dma_start(ut[:], u[rows, :])
            vt = io_pool.tile([P, D], f32, tag="vt")
            nc.sync.dma_start(vt[:], v[rows, :])
            nc.sync.dma_start(ou[rows, :], ut[:])
            nc.sync.dma_start(ov[rows, :], vt[:])

    def body_fused(tc, io_pool, sum_pool):
        # One 4 MiB DMA per tensor covering both 128-row blocks side by
        # side in the free dim; 3D-AP reduce produces both block sums in
        # one instruction.
        u2 = u.rearrange("(n p) d -> p n d", p=P)
        v2 = v.rearrange("(n p) d -> p n d", p=P)
        ou2 = ou.rearrange("(n p) d -> p n d", p=P)
        ov2 = ov.rearrange("(n p) d -> p n d", p=P)
        W = N_BLOCKS * D

        ut = io_pool.tile([P, W], f32, tag="ut")
        nc.sync.dma_start(
            ut[:].rearrange("p (n d) -> p n d", d=D), u2[:, :, :]
        )
        vt = io_pool.tile([P, W], f32, tag="vt")
        nc.sync.dma_start(
            vt[:].rearrange("p (n d) -> p n d", d=D), v2[:, :, :]
        )

        us = sum_pool.tile([P, N_BLOCKS], f32, tag="us")
        nc.vector.reduce_sum(
            us[:], ut[:].rearrange("p (n d) -> p n d", d=D), axis=mybir.AxisListType.X
        )
        vs = sum_pool.tile([P, N_BLOCKS], f32, tag="vs")
        nc.vector.reduce_sum(
            vs[:], vt[:].rearrange("p (n d) -> p n d", d=D), axis=mybir.AxisListType.X
        )

        for blk in range(N_BLOCKS):
            cols = slice(blk * D, (blk + 1) * D)
            nc.scalar.activation(
                ut[:, cols],
                ut[:, cols],
                mybir.ActivationFunctionType.Copy,
                scale=vs[:, blk : blk + 1],
            )
            nc.vector.tensor_scalar_mul(
                vt[:, cols], vt[:, cols], us[:, blk : blk + 1]
            )
        nc.sync.dma_start(
            ou2[:, :, :], ut[:].rearrange("p (n d) -> p n d", d=D)
        )
        nc.sync.dma_start(
            ov2[:, :, :], vt[:].rearrange("p (n d) -> p n d", d=D)
        )

    def body_memcpy_split(tc, io_pool, sum_pool):
        # Same traffic in 1 MiB chunks across more queue slots.
        H = D // 2
        for blk in range(N_BLOCKS):
            rows = slice(blk * P, (blk + 1) * P)
            ut = io_pool.tile([P, D], f32, tag="ut")
            vt = io_pool.tile([P, D], f32, tag="vt")
            for c in range(2):
                cols = slice(c * H, (c + 1) * H)
                nc.sync.dma_start(ut[:, cols], u[rows, cols])
                nc.sync.dma_start(vt[:, cols], v[rows, cols])
            for c in range(2):
                cols = slice(c * H, (c + 1) * H)
                nc.sync.dma_start(ou[rows, cols], ut[:, cols])
                nc.sync.dma_start(ov[rows, cols], vt[:, cols])

    def body_inplace(tc, io_pool, sum_pool):
        # Same as base but scales in place: 2 live [P, D] tags instead of
        # 4, leaving room for bufs=3.
        for blk in range(N_BLOCKS):
            rows = slice(blk * P, (blk + 1) * P)
            ut = io_pool.tile([P, D], f32, tag="ut")
            nc.sync.dma_start(ut[:], u[rows, :])
            vt = io_pool.tile([P, D], f32, tag="vt")
            nc.sync.dma_start(vt[:], v[rows, :])

            us = sum_pool.tile([P, 1], f32, tag="us")
            nc.vector.reduce_sum(us[:], ut[:], axis=mybir.AxisListType.X)
            vs = sum_pool.tile([P, 1], f32, tag="vs")
            nc.vector.reduce_sum(vs[:], vt[:], axis=mybir.AxisListType.X)

            nc.scalar.activation(
                ut[:], ut[:], mybir.ActivationFunctionType.Copy, scale=vs[:]
            )
            nc.vector.tensor_scalar_mul(vt[:], vt[:], us[:])

            nc.sync.dma_start(ou[rows, :], ut[:])
            nc.sync.dma_start(ov[rows, :], vt[:])

    def body_2q(tc, io_pool, sum_pool):
        # Loads on the SP HWDGE queue, stores on the ACT HWDGE queue:
        # directional queue split to overlap reads and writes at the HBM.
        for blk in range(N_BLOCKS):
            rows = slice(blk * P, (blk + 1) * P)
            ut = io_pool.tile([P, D], f32, tag="ut")
            nc.sync.dma_start(ut[:], u[rows, :])
            vt = io_pool.tile([P, D], f32, tag="vt")
            nc.sync.dma_start(vt[:], v[rows, :])

            us = sum_pool.tile([P, 1], f32, tag="us")
            nc.vector.reduce_sum(us[:], ut[:], axis=mybir.AxisListType.X)
            vs = sum_pool.tile([P, 1], f32, tag="vs")
            nc.vector.reduce_sum(vs[:], vt[:], axis=mybir.AxisListType.X)

            out_u = io_pool.tile([P, D], f32, tag="out_u")
            nc.scalar.activation(
                out_u[:], ut[:], mybir.ActivationFunctionType.Copy, scale=vs[:]
            )
            out_v = io_pool.tile([P, D], f32, tag="out_v")
            nc.vector.tensor_scalar_mul(out_v[:], vt[:], us[:])

            nc.scalar.dma_start(ou[rows, :], out_u[:])
            nc.scalar.dma_start(ov[rows, :], out_v[:])

    def body_3q(tc, io_pool, sum_pool):
        # Loads on SP, out_user stores on ACT, out_image stores on SWDGE
        # (gpsimd): three DMA paths.
        for blk in range(N_BLOCKS):
            rows = slice(blk * P, (blk + 1) * P)
            ut = io_pool.tile([P, D], f32, tag="ut")
            nc.sync.dma_start(ut[:], u[rows, :])
            vt = io_pool.tile([P, D], f32, tag="vt")
            nc.sync.dma_start(vt[:], v[rows, :])

            us = sum_pool.tile([P, 1], f32, tag="us")
            nc.vector.reduce_sum(us[:], ut[:], axis=mybir.AxisListType.X)
            vs = sum_pool.tile([P, 1], f32, tag="vs")
            nc.vector.reduce_sum(vs[:], vt[:], axis=mybir.AxisListType.X)

            out_u = io_pool.tile([P, D], f32, tag="out_u")
            nc.scalar.activation(
                out_u[:], ut[:], mybir.ActivationFunctionType.Copy, scale=vs[:]
            )
            out_v = io_pool.tile([P, D], f32, tag="out_v")
            nc.vector.tensor_scalar_mul(out_v[:], vt[:], us[:])

            nc.scalar.dma_start(ou[rows, :], out_u[:])
            nc.gpsimd.dma_start(ov[rows, :], out_v[:])

    def body_2q_dve(tc, io_pool, sum_pool):
        # Loads on SP, stores on ACT, ALL compute on DVE so the ACT engine
        # is a pure store-DMA issuer (no act/store serialization).
        for blk in range(N_BLOCKS):
            rows = slice(blk * P, (blk + 1) * P)
            ut = io_pool.tile([P, D], f32, tag="ut")
            nc.sync.dma_start(ut[:], u[rows, :])
            vt = io_pool.tile([P, D], f32, tag="vt")
            nc.sync.dma_start(vt[:], v[rows, :])

            us = sum_pool.tile([P, 1], f32, tag="us")
            nc.vector.reduce_sum(us[:], ut[:], axis=mybir.AxisListType.X)
            vs = sum_pool.tile([P, 1], f32, tag="vs")
            nc.vector.reduce_sum(vs[:], vt[:], axis=mybir.AxisListType.X)

            out_u = io_pool.tile([P, D], f32, tag="out_u")
            nc.vector.tensor_scalar_mul(out_u[:], ut[:], vs[:])
            out_v = io_pool.tile([P, D], f32, tag="out_v")
            nc.vector.tensor_scalar_mul(out_v[:], vt[:], us[:])

            nc.scalar.dma_start(ou[rows, :], out_u[:])
            nc.scalar.dma_start(ov[rows, :], out_v[:])

    def body_3q_dve(tc, io_pool, sum_pool):
        # Loads on SP, out_user stores on ACT, out_image stores on SWDGE;
        # all compute on DVE.
        for blk in range(N_BLOCKS):
            rows = slice(blk * P, (blk + 1) * P)
            ut = io_pool.tile([P, D], f32, tag="ut")
            nc.sync.dma_start(ut[:], u[rows, :])
            vt = io_pool.tile([P, D], f32, tag="vt")
            nc.sync.dma_start(vt[:], v[rows, :])

            us = sum_pool.tile([P, 1], f32, tag="us")
            nc.vector.reduce_sum(us[:], ut[:], axis=mybir.AxisListType.X)
            vs = sum_pool.tile([P, 1], f32, tag="vs")
            nc.vector.reduce_sum(vs[:], vt[:], axis=mybir.AxisListType.X)

            out_u = io_pool.tile([P, D], f32, tag="out_u")
            nc.vector.tensor_scalar_mul(out_u[:], ut[:], vs[:])
            out_v = io_pool.tile([P, D], f32, tag="out_v")
            nc.vector.tensor_scalar_mul(out_v[:], vt[:], us[:])

            nc.scalar.dma_start(ou[rows, :], out_u[:])
            nc.gpsimd.dma_start(ov[rows, :], out_v[:])

    def body_memcpy_3q(tc, io_pool, sum_pool):
        # Ceiling probe: loads SP, half stores ACT, half stores SWDGE.
        for blk in range(N_BLOCKS):
            rows = slice(blk * P, (blk + 1) * P)
            ut = io_pool.tile([P, D], f32, tag="ut")
            nc.sync.dma_start(ut[:], u[rows, :])
            vt = io_pool.tile([P, D], f32, tag="vt")
            nc.sync.dma_start(vt[:], v[rows, :])
            nc.scalar.dma_start(ou[rows, :], ut[:])
            nc.gpsimd.dma_start(ov[rows, :], vt[:])

    def body_2q_v2(tc, io_pool, sum_pool):
        # Like 2q (loads SP, stores ACT, compute DVE+ACT) but emits both
        # blocks' compute before any store so the ACT stream runs its two
        # act ops before blocking on store-wait sems.
        uts, vts, uss, vss, ous_t, ovs_t = [], [], [], [], [], []
        for blk in range(N_BLOCKS):
            rows = slice(blk * P, (blk + 1) * P)
            ut = io_pool.tile([P, D], f32, tag="ut")
            nc.sync.dma_start(ut[:], u[rows, :])
            vt = io_pool.tile([P, D], f32, tag="vt")
            nc.sync.dma_start(vt[:], v[rows, :])
            uts.append(ut)
            vts.append(vt)
        for blk in range(N_BLOCKS):
            us_ = sum_pool.tile([P, 1], f32, tag="us")
            nc.vector.reduce_sum(us_[:], uts[blk][:], axis=mybir.AxisListType.X)
            vs_ = sum_pool.tile([P, 1], f32, tag="vs")
            nc.vector.reduce_sum(vs_[:], vts[blk][:], axis=mybir.AxisListType.X)
            uss.append(us_)
            vss.append(vs_)
        for blk in range(N_BLOCKS):
            out_u = io_pool.tile([P, D], f32, tag="out_u")
            nc.scalar.activation(
                out_u[:],
                uts[blk][:],
                mybir.ActivationFunctionType.Copy,
                scale=vss[blk][:],
            )
            ous_t.append(out_u)
            out_v = io_pool.tile([P, D], f32, tag="out_v")
            nc.vector.tensor_scalar_mul(out_v[:], vts[blk][:], uss[blk][:])
            ovs_t.append(out_v)
        for blk in range(N_BLOCKS):
            rows = slice(blk * P, (blk + 1) * P)
            nc.scalar.dma_start(ou[rows, :], ous_t[blk][:])
            nc.scalar.dma_start(ov[rows, :], ovs_t[blk][:])

    def body_2q_swap(tc, io_pool, sum_pool):
        # Loads on ACT (pure submissions, no waits), stores on SP; compute
        # split DVE + ACT as in base.
        for blk in range(N_BLOCKS):
            rows = slice(blk * P, (blk + 1) * P)
            ut = io_pool.tile([P, D], f32, tag="ut")
            nc.scalar.dma_start(ut[:], u[rows, :])
            vt = io_pool.tile([P, D], f32, tag="vt")
            nc.scalar.dma_start(vt[:], v[rows, :])

            us = sum_pool.tile([P, 1], f32, tag="us")
            nc.vector.reduce_sum(us[:], ut[:], axis=mybir.AxisListType.X)
            vs = sum_pool.tile([P, 1], f32, tag="vs")
            nc.vector.reduce_sum(vs[:], vt[:], axis=mybir.AxisListType.X)

            out_u = io_pool.tile([P, D], f32, tag="out_u")
            nc.scalar.activation(
                out_u[:], ut[:], mybir.ActivationFunctionType.Copy, scale=vs[:]
            )
            out_v = io_pool.tile([P, D], f32, tag="out_v")
            nc.vector.tensor_scalar_mul(out_v[:], vt[:], us[:])

            nc.sync.dma_start(ou[rows, :], out_u[:])
            nc.sync.dma_start(ov[rows, :], out_v[:])

    def body_2q_bal(tc, io_pool, sum_pool):
        # Loads SP, stores ACT; compute rebalanced: us-sum comes free from
        # an ACT scaled-copy's accum_out, halving DVE's reduce load (DVE
        # reduce+drain is the most expensive op chain).
        for blk in range(N_BLOCKS):
            rows = slice(blk * P, (blk + 1) * P)
            ut = io_pool.tile([P, D], f32, tag="ut")
            nc.sync.dma_start(ut[:], u[rows, :])
            vt = io_pool.tile([P, D], f32, tag="vt")
            nc.sync.dma_start(vt[:], v[rows, :])

            us = sum_pool.tile([P, 1], f32, tag="us")
            scratch = io_pool.tile([P, D], f32, tag="scratch")
            nc.scalar.activation(
                scratch[:],
                ut[:],
                mybir.ActivationFunctionType.Copy,
                accum_out=us[:],
            )
            vs = sum_pool.tile([P, 1], f32, tag="vs")
            nc.vector.reduce_sum(vs[:], vt[:], axis=mybir.AxisListType.X)

            out_u = io_pool.tile([P, D], f32, tag="out_u")
            nc.scalar.activation(
                out_u[:], ut[:], mybir.ActivationFunctionType.Copy, scale=vs[:]
            )
            out_v = io_pool.tile([P, D], f32, tag="out_v")
            nc.vector.tensor_scalar_mul(out_v[:], vt[:], us[:])

            nc.scalar.dma_start(ou[rows, :], out_u[:])
            nc.scalar.dma_start(ov[rows, :], out_v[:])

    def body_memcpy_2q(tc, io_pool, sum_pool):
        # Ceiling probe with the directional 2-queue split.
        for blk in range(N_BLOCKS):
            rows = slice(blk * P, (blk + 1) * P)
            ut = io_pool.tile([P, D], f32, tag="ut")
            nc.sync.dma_start(ut[:], u[rows, :])
            vt = io_pool.tile([P, D], f32, tag="vt")
            nc.sync.dma_start(vt[:], v[rows, :])
            nc.scalar.dma_start(ou[rows, :], ut[:])
            nc.scalar.dma_start(ov[rows, :], vt[:])

    bodies = {
        "base": body_base,
        "memcpy": body_memcpy,
        "memcpy_split": body_memcpy_split,
        "memcpy_2q": body_memcpy_2q,
        "memcpy_3q": body_memcpy_3q,
        "2q_dve": body_2q_dve,
        "3q_dve": body_3q_dve,
        "2q_v2": body_2q_v2,
        "2q_swap": body_2q_swap,
        "2q_bal": body_2q_bal,
        "fused": body_fused,
        "inplace": body_inplace,
        "2q": body_2q,
        "3q": body_3q,
    }
    body = bodies[variant]

    with tile.TileContext(nc) as tc:
        with (
            tc.tile_pool(name="io", bufs=bufs) as io_pool,
            tc.tile_pool(name="sums", bufs=bufs) as sum_pool,
        ):
            with tc.For_i(0, iters, 1):
                for _rep in range(unroll):
                    body(tc, io_pool, sum_pool)

    nc.compile()
    return nc


def _get_loop_runner(iters, unroll=4, variant="base", bufs=2):
    key = ("loop", iters, unroll, variant, bufs)
    if key not in _CACHE:
        _CACHE[key] = _make_runner(_build_loop(iters, unroll, variant, bufs))
    return _CACHE[key]


def _build_raw(passes=1):
    """Raw bacc kernel with manual semaphores — no TileContext, so no Tile
    preamble (memset/drain block) and no kernel-tail EVSEM butterfly
    (~9-17 us per NEFF). Same dataflow as _build.

    `passes` > 1 statically unrolls repeat passes with parity double
    buffering (two SBUF tile sets) for steady-state timing measurements.

    Dependency scheme per pass rep (set s = rep % 2, k = rep // 2):
      - per-tile load sems in_u/in_v (+16 per use) gate compute;
      - v_sem counts 6 vector ops/pass, s_sem 2 scalar ops/pass;
      - per-tile store sems ou_done/ov_done (+16) gate the next reuse of
        the same tile set (WAR), and the final end-of-program waits.
    In-place scaling: ACT overwrites ut (needs v_sem>=6r+2: both its scale
    vs and the us reduce that read ut are done), DVE overwrites vt.

    DMA queues are directional: SP issues all loads (qSPDynamicHW), ACT
    issues all stores (qActDynamicHW) right after its own act op — in a
    single shot, block-0 stores overlap block-1 loads on the other queue.
    Same-engine hazards (DGE store reading a tile the issuing ACT just
    wrote; DVE mul reading us its own reduce produced) are covered by
    self-waits on s_sem/v_sem.
    """
    from concourse import bacc, mybir

    nc = bacc.Bacc(
        "TRN2",
        target_bir_lowering=False,
        debug=False,
        enable_asserts=False,
        num_devices=N_CORES,
    )
    f32 = mybir.dt.float32

    u = nc.dram_tensor("user_attributes", [ROWS, D], f32, kind="ExternalInput").ap()
    v = nc.dram_tensor("image_attributes", [ROWS, D], f32, kind="ExternalInput").ap()
    ou = nc.dram_tensor("out_user", [ROWS, D], f32, kind="ExternalOutput").ap()
    ov = nc.dram_tensor("out_image", [ROWS, D], f32, kind="ExternalOutput").ap()

    SETS = 2 if passes > 1 else 1
    ut = [
        [nc.alloc_sbuf_tensor(f"ut{s}_{b}", [P, D], f32).ap() for b in range(N_BLOCKS)]
        for s in range(SETS)
    ]
    vt = [
        [nc.alloc_sbuf_tensor(f"vt{s}_{b}", [P, D], f32).ap() for b in range(N_BLOCKS)]
        for s in range(SETS)
    ]
    us = [
        [nc.alloc_sbuf_tensor(f"us{s}_{b}", [P, 1], f32).ap() for b in range(N_BLOCKS)]
        for s in range(SETS)
    ]
    vs = [
        [nc.alloc_sbuf_tensor(f"vs{s}_{b}", [P, 1], f32).ap() for b in range(N_BLOCKS)]
        for s in range(SETS)
    ]

    in_u = [[nc.alloc_semaphore(f"in_u{s}_{b}") for b in range(N_BLOCKS)] for s in range(SETS)]
    in_v = [[nc.alloc_semaphore(f"in_v{s}_{b}") for b in range(N_BLOCKS)] for s in range(SETS)]
    ou_done = [[nc.alloc_semaphore(f"ou{s}_{b}") for b in range(N_BLOCKS)] for s in range(SETS)]
    ov_done = [[nc.alloc_semaphore(f"ov{s}_{b}") for b in range(N_BLOCKS)] for s in range(SETS)]
    v_sem = nc.alloc_semaphore("v_sem")
    s_sem = nc.alloc_semaphore("s_sem")

    def sk(rep):
        return (rep % SETS, rep // SETS)

    def uses(s):
        return (passes + SETS - 1 - s) // SETS if SETS > 1 else passes

    with nc.Block() as block:

        @block.sync
        def _(sync):
            for rep in range(passes):
                s, k = sk(rep)
                for b in range(N_BLOCKS):
                    rows = slice(b * P, (b + 1) * P)
                    if k > 0:
                        sync.wait_ge(ou_done[s][b], 16 * k)
                    sync.dma_start(ut[s][b][:], u[rows, :]).then_inc(in_u[s][b], 16)
                    if k > 0:
                        sync.wait_ge(ov_done[s][b], 16 * k)
                    sync.dma_start(vt[s][b][:], v[rows, :]).then_inc(in_v[s][b], 16)
            for s in range(SETS):
                n = uses(s)
                if n:
                    for b in range(N_BLOCKS):
                        sync.wait_ge(in_u[s][b], 16 * n)
                        sync.wait_ge(in_v[s][b], 16 * n)

        @block.vector
        def _(vector):
            from concourse import mybir as mb

            for rep in range(passes):
                s, k = sk(rep)
                for b in range(N_BLOCKS):
                    vector.wait_ge(in_u[s][b], 16 * (k + 1))
                    nc.vector.reduce_sum(
                        us[s][b][:], ut[s][b][:], axis=mb.AxisListType.X
                    ).then_inc(v_sem, 1)
                    vector.wait_ge(in_v[s][b], 16 * (k + 1))
                    nc.vector.reduce_sum(
                        vs[s][b][:], vt[s][b][:], axis=mb.AxisListType.X
                    ).then_inc(v_sem, 1)
                    # Same-engine RAW on us through the DVE pipe still needs
                    # an explicit sem wait (deep pipeline hazard).
                    vector.wait_ge(v_sem, 6 * rep + 3 * b + 1)
                    nc.vector.tensor_scalar_mul(
                        vt[s][b][:], vt[s][b][:], us[s][b][:]
                    ).then_inc(v_sem, 1)

        @block.scalar
        def _(scalar):
            from concourse import mybir as mb

            for rep in range(passes):
                s, k = sk(rep)
                for b in range(N_BLOCKS):
                    rows = slice(b * P, (b + 1) * P)
                    scalar.wait_ge(in_u[s][b], 16 * (k + 1))
                    scalar.wait_ge(v_sem, 6 * rep + 3 * b + 2)
                    nc.scalar.activation(
                        ut[s][b][:],
                        ut[s][b][:],
                        mb.ActivationFunctionType.Copy,
                        scale=vs[s][b][:],
                    ).then_inc(s_sem, 1)
                    # Self-wait: the store's DGE must not read ut until the
                    # act above has fully retired.
                    scalar.wait_ge(s_sem, 2 * rep + b + 1)
                    scalar.dma_start(ou[rows, :], ut[s][b][:]).then_inc(
                        ou_done[s][b], 16
                    )
                    scalar.wait_ge(v_sem, 6 * rep + 3 * b + 3)
                    scalar.dma_start(ov[rows, :], vt[s][b][:]).then_inc(
                        ov_done[s][b], 16
                    )
            for s in range(SETS):
                n = uses(s)
                if n:
                    for b in range(N_BLOCKS):
                        scalar.wait_ge(ou_done[s][b], 16 * n)
                        scalar.wait_ge(ov_done[s][b], 16 * n)

    nc.compile()
    return nc


def _get_raw_runner(passes=1):
    key = ("raw", passes)
    if key not in _CACHE:
        _CACHE[key] = _make_runner(_build_raw(passes))
    return _CACHE[key]


def _make_runner(nc):
    """Jitted 8-core sharded executor for a compiled Bacc program. Mirrors
    concourse.bass2jax.run_bass_via_pjrt's multi-core path, but cached so
    repeat invocations skip retrace/recompile."""
    import jax
    from jax.experimental.shard_map import shard_map
    from jax.sharding import Mesh, PartitionSpec

    from concourse import bass2jax, mybir

    bass2jax.install_neuronx_cc_hook()

    partition_name = nc.partition_id_tensor.name if nc.partition_id_tensor else None
    in_names, out_names, out_avals = [], [], []
    for alloc in nc.m.functions[0].allocations:
        if not isinstance(alloc, mybir.MemoryLocationSet):
            continue
        name = alloc.memorylocations[0].name
        if alloc.kind == "ExternalInput":
            if name != partition_name:
                in_names.append(name)
        elif alloc.kind == "ExternalOutput":
            out_names.append(name)
            out_avals.append(
                jax.core.ShapedArray(
                    tuple(alloc.tensor_shape), mybir.dt.np(alloc.dtype)
                )
            )
    all_in_names = list(in_names) + list(out_names)
    if partition_name is not None:
        all_in_names.append(partition_name)
    all_in_names = tuple(all_in_names)

    def _body(*args):
        operands = list(args)
        if partition_name is not None:
            operands.append(bass2jax.partition_id_tensor())
        outs = bass2jax._bass_exec_p.bind(
            *operands,
            out_avals=tuple(out_avals),
            in_names=all_in_names,
            out_names=tuple(out_names),
            lowering_input_output_aliases=(),
            sim_require_finite=True,
            sim_require_nnan=True,
            nc=nc,
        )
        return tuple(outs)

    devices = jax.devices()[:N_CORES]
    assert len(devices) == N_CORES
    mesh = Mesh(np.asarray(devices), ("core",))
    fn = jax.jit(
        shard_map(
            _body,
            mesh=mesh,
            in_specs=(PartitionSpec("core"),) * (len(in_names) + len(out_names)),
            out_specs=(PartitionSpec("core"),) * len(out_names),
            check_rep=False,
        ),
        keep_unused=True,
    )
    return fn, in_names, out_names


def _get_runner(repeat=1):
    key = ("runner", repeat)
    if key not in _CACHE:
        _CACHE[key] = _make_runner(_build(repeat))
    return _CACHE[key]


def _prep(user_attributes, image_attributes):
    ua = np.ascontiguousarray(np.asarray(user_attributes, dtype=np.float32))
    ia = np.ascontiguousarray(np.asarray(image_attributes, dtype=np.float32))
    assert ua.shape == (B, D) and ia.shape == (B, D)
    return {"user_attributes": ua, "image_attributes": ia}


def kernel(user_attributes, image_attributes):
    import jax

    # Production path: the raw (non-Tile) kernel — same body dataflow, but
    # no Tile preamble/kernel-tail EVSEM butterfly (~9-17 us/NEFF saved)
    # and directional DMA queues (loads on SP, stores on ACT).
    fn, in_names, out_names = _get_raw_runner(1)
    if "zeros" not in _CACHE:
        # Output operands for the custom call (not donated, so they stay
        # valid across calls; the kernel writes every output element).
        _CACHE["zeros"] = [
            jax.device_put(np.zeros((B, D), np.float32)) for _ in out_names
        ]
    named = _prep(user_attributes, image_attributes)
    args = [named[n] for n in in_names] + _CACHE["zeros"]
    try:
        outs = fn(*args)
        outs = [np.asarray(o) for o in outs]
    except Exception:
        # Retry for transient relay/device hiccups. If the mesh desynced
        # (NRT_EXEC_UNIT_UNRECOVERABLE wedges the backend for the process),
        # tear down the PJRT backend and rebuild everything once.
        try:
            outs = fn(*args)
            outs = [np.asarray(o) for o in outs]
        except Exception:
            import jax._src.xla_bridge as xb

            jax.clear_caches()
            xb._clear_backends()
            _CACHE.clear()
            fn, in_names, out_names = _get_raw_runner(1)
            _CACHE["zeros"] = [
                jax.device_put(np.zeros((B, D), np.float32)) for _ in out_names
            ]
            args = [named[n] for n in in_names] + _CACHE["zeros"]
            outs = fn(*args)
            outs = [np.asarray(o) for o in outs]
    by_name = dict(zip(out_names, outs))
    return (by_name["out_user"], by_name["out_image"])
